# revision 26
# baseline (speedup 1.0000x reference)
"""Trainium2 kernel for nn_AdaptivePatchEmbedding.

Strategy (data-parallel over batch, 32 samples / 8 cores = 4 each):
  - Device: the heavy multi-scale CNN feature extractor. The three branches
    (per-channel time conv ks=9/25/49 followed by a channel-collapsing conv)
    algebraically fuse into ONE Conv1d with 22 in-channels, 120 out-channels
    and 49 taps:  W[o,c,k] = sum_i w2[o,i,c] * w1[i,k].
    On the PE this runs as 10 accumulating matmuls (tap groups of 5, K=110)
    per [120,500] output tile, against an SBUF copy of x replicated 5x with
    per-replica time shifts so every tap group is a plain sliced view.
    BN (eval affine) + ELU fused in the epilogue.
  - Host: STFT activity detector -> adaptive grid (tiny, control-flow heavy),
    ragged segment avg+max pooling over the device features, projection.
"""

import numpy as np

B, C_IN, T = 32, 22, 5000
EMB, P = 64, 100
HOP, NFFT, SR = 25, 128, 250
MIN_SZ, MAX_SZ = 10, 100
OC = 40
SCALES = (9, 25, 49)
FC = OC * len(SCALES)
NCORES = 8
BL = B // NCORES
KMAX = 49
PADL = 24
TPAD = T + 2 * PADL          # 5048 valid padded cols
TP = TPAD + 8                # 5056, slack so the shifted-replica DMA stays in bounds
NG = 10                      # tap groups of 5 (last group has 4 real taps)
TT = 500                     # time tile (<=512 psum bank)
EPS = 1e-5

_freqs = np.arange(NFFT // 2 + 1) * SR / NFFT
MU_IDX = np.nonzero((_freqs >= 8) & (_freqs <= 13))[0]
BETA_IDX = np.nonzero((_freqs >= 13) & (_freqs <= 30))[0]

_CACHE = {}


# ---------------------------------------------------------------- device ----
def _patch_drain():
    """The walrus build here caps sync-waits per instruction; Tile's tail
    drain wants one wait per active proc. Pre-observe each proc with a
    single-wait NOP on the sync engine so the drain's waits are elided."""
    import concourse.tile as tile
    if getattr(tile.TileContext, "_drain_split_patched", False):
        return
    from concourse.vector_clock import ScopedClock, VectorClock
    from concourse.tile_sem_assignment import N_PROCS
    _orig = tile.TileContext._drain_and_barrier

    def _split(self, tick_clock, wait_clock):
        gc = tick_clock.global_clock
        for p in range(N_PROCS):
            if gc[p] > 0:
                single = VectorClock(
                    [gc[i] if i == p else 0 for i in range(N_PROCS)])
                nop = self.nc.sync.nop()
                wait_clock.add_sem_waits(nop.ins, ScopedClock({None: single}))
        # replicate _drain_and_barrier minus its multi-wait drain: the
        # single-wait NOPs above already gate the SP queue on every proc
        self.nc.sync.drain()
        self.nc.all_engine_barrier()
        popped = self.nc._tile_sem_poison_stack.pop()
        assert popped is self._sem_poison
        self.nc.clear_and_free_semaphores(list(self.sems.allocated().values()))
        self.nc.all_engine_barrier()

    tile.TileContext._drain_and_barrier = _split
    tile.TileContext._drain_split_patched = True


def _build_nc():
    import concourse.bass as bass
    import concourse.tile as tile
    from concourse import mybir
    _patch_drain()

    nc = bass.Bass("TRN2")
    NIN = NG * FC + BL * TPAD
    xin = nc.dram_tensor("xin", [111, NIN], mybir.dt.bfloat16, kind="ExternalInput")
    out = nc.dram_tensor("out", [BL, FC, T], mybir.dt.float32, kind="ExternalOutput")
    W0 = NG * FC

    with tile.TileContext(nc) as tc:
        with tc.tile_pool(name="xi", bufs=1) as xp, \
             tc.tile_pool(name="ps", bufs=6, space="PSUM") as pp, \
             tc.tile_pool(name="dps", bufs=1, space="PSUM") as dp, \
             tc.tile_pool(name="ob", bufs=1) as ob:
            # ONE input DMA: [weights | per-sample shifted x replicas | ones]
            xs = xp.tile([111, NIN], mybir.dt.bfloat16)
            nc.sync.dma_start(out=xs[:, :], in_=xin[:, :])

            # dummy 1x1 matmul absorbs the input-DMA wait on PE, so every
            # real matmul carries only its single PSUM-WAR wait (HW allows
            # one sync-wait per compute instruction)
            dps = dp.tile([1, 1], mybir.dt.float32)
            nc.tensor.matmul(dps[:, :], lhsT=xs[0:1, 0:1],
                             rhs=xs[0:1, 0:1], start=True, stop=True)

            # ONE output buffer written by DVE copies, one dma at the end
            obuf = ob.tile([FC, BL * T], mybir.dt.float32)
            for s in range(BL):
                x0 = W0 + s * TPAD
                for it in range(T // TT):
                    t0 = it * TT
                    ps = pp.tile([FC, TT], mybir.dt.float32)
                    for g in range(NG):
                        nc.tensor.matmul(
                            ps[:, :],
                            lhsT=xs[:111, g * FC:(g + 1) * FC],
                            rhs=xs[:, x0 + t0 + 5 * g: x0 + t0 + 5 * g + TT],
                            start=(g == 0),
                            stop=(g == NG - 1),
                        )
                    nc.vector.tensor_copy(obuf[:, s * T + t0:s * T + t0 + TT], ps[:, :])
            # out[s, o, t] <- obuf[o, s*T + t]
            odst = bass.AP(tensor=out[0].tensor, offset=out[0].offset,
                           ap=[[T, FC], [FC * T, BL], [1, T]])
            nc.sync.dma_start(out=odst, in_=obuf[:, :])
    return nc


def _run_device(xpad_bf, wt_bf, scale, shift, trace=False):
    from concourse import bass_utils
    if "nc" not in _CACHE:
        _CACHE["nc"] = _build_nc()
    nc = _CACHE["nc"]
    in_maps = []
    for c in range(NCORES):
        blk = [wt_bf] + [xpad_bf[c * BL + s] for s in range(BL)]
        in_maps.append({"xin": np.ascontiguousarray(np.concatenate(blk, axis=1))})
    try:
        res = bass_utils.run_bass_kernel_spmd(
            nc, in_maps, core_ids=list(range(NCORES)), trace=trace)
    except Exception:
        import time
        time.sleep(5)  # transient NRT device hiccups recover on retry
        res = bass_utils.run_bass_kernel_spmd(
            nc, in_maps, core_ids=list(range(NCORES)), trace=trace)
    feats = np.concatenate(
        [np.asarray(r["out"]).astype(np.float32) for r in res.results], axis=0)
    return feats, res


# ------------------------------------------------------------------ host ----
def _sigmoid(v):
    return 1.0 / (1.0 + np.exp(-v))


def _host_activity(x, a_w1, a_b1, a_g, a_bt, a_m, a_v, a_w2, a_b2):
    pad = NFFT // 2
    xp = np.pad(x, ((0, 0), (0, 0), (pad, pad)), mode="reflect")
    nfr = T // HOP
    idx = np.arange(nfr)[:, None] * HOP + np.arange(NFFT)[None, :]
    win = (0.5 * (1.0 - np.cos(2.0 * np.pi * np.arange(NFFT) / NFFT))).astype(np.float32)
    frames = xp[..., idx] * win
    spec = np.fft.rfft(frames.astype(np.float32), axis=-1)
    pw = (spec.real ** 2 + spec.imag ** 2).astype(np.float32)
    comb = pw[..., MU_IDX].mean(-1) + pw[..., BETA_IDX].mean(-1)     # [B,C,nfr]
    cpad = np.pad(comb, ((0, 0), (0, 0), (1, 1)))
    stk = np.stack([cpad[:, :, k:k + nfr] for k in range(3)], axis=-1)  # [B,C,nfr,3]
    a1 = np.einsum("oik,bitk->bot", a_w1, stk) + a_b1[None, :, None]
    a1 = (a1 - a_m[None, :, None]) / np.sqrt(a_v + EPS)[None, :, None] \
        * a_g[None, :, None] + a_bt[None, :, None]
    a1 = np.maximum(a1, 0.0)
    a2 = np.einsum("oi,bit->bot", a_w2[:, :, 0], a1)[:, 0] + a_b2[0]
    activity = _sigmoid(a2).astype(np.float32)                        # [B,nfr]
    # linear interp to P points (align_corners=False)
    L = activity.shape[-1]
    pos = np.clip((np.arange(P, dtype=np.float32) + 0.5) * (L / P) - 0.5, 0.0, L - 1.0)
    lo = np.floor(pos).astype(np.int32)
    hi = np.minimum(lo + 1, L - 1)
    wgt = (pos - lo).astype(np.float32)
    act_i = activity[:, lo] * (1.0 - wgt) + activity[:, hi] * wgt
    return act_i.astype(np.float32)


def _adaptive_grid(act):
    act = act.astype(np.float32)
    w = (1.0 / (act + np.float32(1e-6))).astype(np.float32)
    w = w / w.sum(axis=1, keepdims=True, dtype=np.float32)
    gs = np.clip(w * np.float32(T), MIN_SZ, MAX_SZ).astype(np.float32)
    gs = gs * (np.float32(T) / gs.sum(axis=1, keepdims=True, dtype=np.float32))
    gsr = np.round(gs).astype(np.int32)
    gsr[:, -1] += np.int32(T) - gsr.sum(axis=1)
    gsf = np.clip(gsr, MIN_SZ, MAX_SZ)
    gsf[:, -1] += np.int32(T) - gsf.sum(axis=1)
    return np.where(gsf < MIN_SZ, MIN_SZ, gsf).astype(np.int32)


def _fold_weights(inputs):
    Wall = np.zeros((FC, C_IN, KMAX), np.float32)
    scale = np.zeros(FC, np.float32)
    shift = np.zeros(FC, np.float32)
    for i, ks in enumerate(SCALES):
        w1 = inputs[f"ms{i}_w1"].reshape(OC, ks)
        b1 = inputs[f"ms{i}_b1"]
        w2 = inputs[f"ms{i}_w2"][..., 0]                  # [o,i,c]
        b2 = inputs[f"ms{i}_b2"]
        Wf = np.einsum("oic,ik->ock", w2, w1)
        off = (KMAX - ks) // 2
        sl = slice(i * OC, (i + 1) * OC)
        Wall[sl, :, off:off + ks] = Wf
        beta = w2.sum(axis=2) @ b1 + b2                   # conv bias folded
        s_ = inputs[f"ms{i}_g"] / np.sqrt(inputs[f"ms{i}_v"] + EPS)
        scale[sl] = s_
        shift[sl] = (beta - inputs[f"ms{i}_m"]) * s_ + inputs[f"ms{i}_bt"]
    Wall *= scale[:, None, None]          # fold BN scale into conv weights
    wt = np.zeros((111, NG * FC), np.float32)
    for g in range(NG):
        for kl in range(5):
            K = 5 * g + kl
            if K < KMAX:
                wt[kl * C_IN:(kl + 1) * C_IN, g * FC:(g + 1) * FC] = Wall[:, :, K].T
    wt[110, 0:FC] = shift                  # bias row, applied once via g=0
    return wt, scale, shift


def _pool_project(feats, grid, proj_w, proj_b):
    ends = np.cumsum(grid.astype(np.int64), axis=1)
    starts = ends - grid
    tix = np.arange(T)
    fT = feats.transpose(0, 2, 1)                          # [B,T,FC]
    pooled = np.zeros((B, P + 1, FC), np.float32)
    for b in range(B):
        sb = np.searchsorted(ends[b], tix, side="right")
        cnt = np.bincount(sb, minlength=P + 1)
        if np.all(np.diff(sb) >= 0):
            offs = np.concatenate([[0], np.cumsum(cnt)])
            for p in range(P + 1):
                a, e = offs[p], offs[p + 1]
                if e > a:
                    blk = fT[b, a:e]
                    pooled[b, p] = blk.sum(0) / (e - a) + blk.max(0)
        else:  # pathological non-monotone ends: exact but slow path
            sums = np.zeros((P + 1, FC), np.float32)
            mx = np.full((P + 1, FC), -np.inf, np.float32)
            np.add.at(sums, sb, fT[b])
            np.maximum.at(mx, sb, fT[b])
            nz = cnt > 0
            pooled[b, nz] = sums[nz] / cnt[nz, None] + mx[nz]
    pooled = pooled[:, :P]
    valid = (ends[:, :P] <= T) & (starts[:, :P] < ends[:, :P])
    pooled = np.where(valid[..., None], pooled, 0.0).astype(np.float32)
    emb = pooled @ proj_w[..., 0].T + proj_b
    return emb.astype(np.float32)


def _build_xrep(x):
    import ml_dtypes
    xpad = np.zeros((B, C_IN, TP), np.float32)
    xpad[:, :, PADL:PADL + T] = x
    import ml_dtypes
    xrep = np.empty((B, 111, TPAD), ml_dtypes.bfloat16)
    for kl in range(5):
        xrep[:, kl * C_IN:(kl + 1) * C_IN, :] = xpad[:, :, kl:kl + TPAD]
    xrep[:, 110, :] = 1.0
    return xrep


def kernel(**inputs):
    import ml_dtypes
    inputs = {k: np.asarray(v) for k, v in inputs.items()}
    x = inputs["x"].astype(np.float32)

    act_i = _host_activity(x, inputs["a_w1"], inputs["a_b1"], inputs["a_g"],
                           inputs["a_bt"], inputs["a_m"], inputs["a_v"],
                           inputs["a_w2"], inputs["a_b2"])
    grid = _adaptive_grid(act_i)

    wt, scale, shift = _fold_weights(inputs)
    import ml_dtypes
    xpad_bf = _build_xrep(x)
    wt_bf = wt.astype(ml_dtypes.bfloat16)

    z, _ = _run_device(xpad_bf, wt_bf, scale, shift)
    feats = np.where(z > 0, z, np.exp(np.minimum(z, 0.0)) - 1.0).astype(np.float32)

    emb = _pool_project(feats, grid, inputs["proj_w"], inputs["proj_b"])
    return emb, grid.astype(np.int32), act_i.astype(np.float32)
